# revision 17
# baseline (speedup 1.0000x reference)
"""Cosine-similarity retrieval kernel for Trainium2 (8 NeuronCores, SPMD).

Computes out[q, n] = cos(query[q], support[n]) for query [2048, 512] and
support [50000, 512], out [2048, 50000] float32 — matching
torch.nn.CosineSimilarity semantics (dots / max(|q|*|s|, 1e-8)).

Strategy:
  * Shard support on the N axis: 8 shards of 6250 rows (zero-padded to 6272 =
    49 blocks of 128). Each core reads its shard plus the replicated query
    set and writes its own [6272, 2048] output block (n-major, i.e. the
    transpose of the final layout); the host trims/transposes/concatenates —
    no device collective needed.
  * Rows are pre-normalized on the host (norms in float64), so the device
    kernel is a pure matmul; the PSUM result IS the cosine.
  * Storage/matmul dtype is fp16 (1 cycle/row on the PE, same as fp32r, but
    weights go through the LDWEIGHTS+FWL path instead of per-matmul fp32
    self-loading). The support block [128d, 128n] is the STATIONARY operand,
    reused across 4 consecutive matmuls that stream the resident query set
    512 columns at a time; with walrus --enable-ldw-opt the LDWEIGHTS for
    repeats is deduped, so weight-load overhead amortizes 4x and prefetches
    into the PE background buffer during the preceding matmuls.
  * PSUM: 4 banks accumulate one n-block over the 4 k-slices (bank = [128,
    512] fp32 = exactly one 2KB bank); the other 4 banks drain the previous
    n-block through ACT/DVE fp32->fp16 copies, so the PE never waits.
  * Output staged fp16 (halves the dominant HBM write traffic; host upcasts;
    ~2.4e-4 extra rel-L2). One store per n-block: 4KB-contiguous per
    partition, and the final store is only 0.5MB so the kernel-exit barrier
    isn't stuck behind a big trailing DMA.
"""

import os

import numpy as np

QN, DN, NN = 2048, 512, 50000
N_CORES = 8
NSH = NN // N_CORES  # 6250 support rows per core
P = 128
KT = DN // P  # 4 contraction slices
NBLK = (NSH + P - 1) // P  # 49 n-blocks per core
NSHP = NBLK * P  # 6272 (22 zero-padded rows, trimmed on host)
QC = 4  # query chunks, each one PSUM bank wide
QW = QN // QC  # 512 fp32 = one full PSUM bank
# n-blocks per DMA slab: small first slab so the first matmul unblocks after
# ~0.3MB of DMA; 1MB slabs after that for 2KB-contiguous packets.
SLAB_BLOCKS = [2, 8, 8, 8, 8, 8, 7]
WARMUP_MMS = 12  # dummy matmuls on memset data: ramp the PE p-state/HAM
# during the startup DMA window so real matmuls start at full clock
SLAB_PREFETCH = 3
EPS = 1e-8

# "fp16" (default), "bf16", or "fp32r": SBUF/DMA storage + matmul dtype.
DT_MODE = os.environ.get("COS_DT_MODE", "fp16")
# Output staging dtype: "fp16" (default) or "fp32".
OUT_MODE = os.environ.get("COS_OUT_DT", "fp16")

_PROGRAM = {}


def _patch_ldw_opt():
    """walrus's LDWEIGHTS dedup (--enable-ldw-opt) is hardcoded off in
    concourse; consecutive matmuls here share weights, so turn it on."""
    from concourse import bass_utils as bu

    if getattr(bu.run_command, "_ldw_patched", False):
        return
    orig = bu.run_command

    def patched(argv, **kwargs):
        if isinstance(argv, list) and "--enable-ldw-opt=false" in argv:
            argv = [
                "--enable-ldw-opt=true" if a == "--enable-ldw-opt=false" else a
                for a in argv
            ]
        return orig(argv, **kwargs)

    patched._ldw_patched = True
    bu.run_command = patched


def _build_program(dt_mode, out_mode):
    import concourse.bass as bass  # noqa: F401
    import concourse.tile as tile
    from concourse import bacc, mybir

    if os.environ.get("COS_LDW_OPT", "1") != "0":
        _patch_ldw_opt()

    store_dt = {
        "fp16": mybir.dt.float16,
        "bf16": mybir.dt.bfloat16,
        "fp32r": mybir.dt.float32r,
    }[dt_mode]
    out_dt = mybir.dt.float16 if out_mode == "fp16" else mybir.dt.float32

    nc = bacc.Bacc(
        "TRN2", target_bir_lowering=False, debug=False, num_devices=N_CORES
    )
    qT = nc.dram_tensor("qT", [DN, QN], store_dt, kind="ExternalInput").ap()
    sT = nc.dram_tensor("sT", [DN, NSHP], store_dt, kind="ExternalInput").ap()
    out = nc.dram_tensor("out", [NSHP, QN], out_dt, kind="ExternalOutput").ap()

    qT3 = qT.rearrange("(k p) q -> p k q", p=P)  # [128, KT, QN]
    sT3 = sT.rearrange("(k p) n -> p k n", p=P)  # [128, KT, NSHP]
    out3 = out.rearrange("(s p) q -> p s q", p=P)  # [128, NBLK, QN]

    slab_off, o = [], 0
    for nb in SLAB_BLOCKS:
        slab_off.append(o)
        o += nb
    assert o == NBLK

    with tile.TileContext(nc) as tc:
        with (
            tc.tile_pool(name="qw", bufs=1) as qpool,
            tc.tile_pool(name="sw", bufs=SLAB_PREFETCH + 1) as spool,
            tc.tile_pool(name="ps", bufs=8, space="PSUM") as pspool,
            tc.tile_pool(name="ostage", bufs=4) as opool,
        ):
            qt = qpool.tile([P, KT, QN], store_dt, name="qt", tag="qt")
            slabs = {}

            def load_slab(si, per_k=False):
                nb = SLAB_BLOCKS[si]
                n0 = slab_off[si] * P
                w = nb * P
                t = spool.tile(
                    [P, KT, 8 * P],
                    store_dt,
                    name=f"s{si}",
                    tag="ss",
                    bufs=SLAB_PREFETCH + 1,
                )
                if per_k:
                    for k in range(KT):
                        nc.sync.dma_start(t[:, k, :w], sT3[:, k, n0 : n0 + w])
                else:
                    nc.sync.dma_start(t[:, :, :w], sT3[:, :, n0 : n0 + w])
                slabs[si] = t

            # Startup: everything stays on the sync queue (spreading over
            # other engines' queues loses: each ring pays its own multi-us
            # cold ramp, and the extra DMA semaphores lengthen the exit
            # drain/clear epilogue).  s0 k-slices interleave with the query
            # k-slices in first-block need-order; the startup is then purely
            # DMA-bandwidth-bound (~2.3MB before the first full block pass).
            nb0 = SLAB_BLOCKS[0]
            w0 = nb0 * P
            t0s = spool.tile(
                [P, KT, 8 * P],
                store_dt,
                name="s0",
                tag="ss",
                bufs=SLAB_PREFETCH + 1,
            )
            for k in range(KT):
                nc.sync.dma_start(t0s[:, k, :w0], sT3[:, k, 0:w0])
                nc.sync.dma_start(qt[:, k, :], qT3[:, k, :])
            slabs[0] = t0s
            for si in range(1, SLAB_PREFETCH):
                load_slab(si)

            # PE warm-up: stream zeros through the array while the first real
            # operands are still in flight, so the p-state/HAM governor is at
            # full clock when the real matmuls start (cold-start matmuls
            # otherwise run ~1.5x slow for the first ~3us of busy time).
            if WARMUP_MMS:
                wu = qpool.tile([P, P + QW], store_dt, name="wu", tag="wu")
                nc.gpsimd.memset(wu[:, :], 0)
                wps = pspool.tile([P, QW], mybir.dt.float32, name="ps", tag="ps")
                for _ in range(WARMUP_MMS):
                    nc.tensor.matmul(
                        wps[:, :],
                        lhsT=wu[:, :P],
                        rhs=wu[:, P : P + QW],
                        start=True,
                        stop=True,
                    )

            copy_idx = 0
            for si, nb in enumerate(SLAB_BLOCKS):
                if si + SLAB_PREFETCH < len(SLAB_BLOCKS):
                    load_slab(si + SLAB_PREFETCH)
                for b in range(nb):
                    sb = slab_off[si] + b
                    last = sb == NBLK - 1
                    pss = [
                        pspool.tile(
                            [P, QW], mybir.dt.float32, name="ps", tag="ps"
                        )
                        for _ in range(QC)
                    ]
                    ot = opool.tile([P, QN], out_dt, name="ot", tag="ot")
                    if not last:
                        # k outer / qc inner: the 4 qc matmuls stream against
                        # one stationary [128, 128] support block, so its
                        # LDWEIGHTS prefetches into the PE background buffer
                        # during the previous k's matmuls.
                        for k in range(KT):
                            wt = slabs[si][:, k, b * P : (b + 1) * P]
                            for qc in range(QC):
                                nc.tensor.matmul(
                                    pss[qc][:, :],
                                    lhsT=wt,
                                    rhs=qt[:, k, qc * QW : (qc + 1) * QW],
                                    start=(k == 0),
                                    stop=(k == KT - 1),
                                )
                        # split PSUM->SBUF downcast copies across ACT/DVE
                        for qc in range(QC):
                            dst = ot[:, qc * QW : (qc + 1) * QW]
                            if copy_idx % 2 == 0:
                                nc.scalar.copy(out=dst, in_=pss[qc][:, :])
                            else:
                                nc.vector.tensor_copy(out=dst, in_=pss[qc][:, :])
                            copy_idx += 1
                        nc.sync.dma_start(out3[:, sb, :], ot[:, :])
                    else:
                        # Final block: 8 chunks of 256 on all 8 banks, chunk
                        # outer so each bank stops as early as possible and
                        # drains copy->store immediately; the kernel-exit
                        # barrier then only waits on a 64KB trailing store
                        # instead of the full 0.5MB block.
                        HW8 = QN // 8  # 256
                        pss8 = [
                            pspool.tile(
                                [P, QW], mybir.dt.float32, name="ps", tag="ps"
                            )
                            for _ in range(4)
                        ]
                        for hc in range(8):
                            ps = (pss + pss8)[hc]
                            for k in range(KT):
                                nc.tensor.matmul(
                                    ps[:, :HW8],
                                    lhsT=slabs[si][:, k, b * P : (b + 1) * P],
                                    rhs=qt[:, k, hc * HW8 : (hc + 1) * HW8],
                                    start=(k == 0),
                                    stop=(k == KT - 1),
                                )
                            dst = ot[:, hc * HW8 : (hc + 1) * HW8]
                            if hc % 2 == 0:
                                nc.scalar.copy(out=dst, in_=ps[:, :HW8])
                            else:
                                nc.vector.tensor_copy(out=dst, in_=ps[:, :HW8])
                            nc.sync.dma_start(
                                out3[:, sb, hc * HW8 : (hc + 1) * HW8], dst
                            )
    nc.compile()
    return nc


def _get_program(dt_mode=None, out_mode=None):
    key = (dt_mode or DT_MODE, out_mode or OUT_MODE)
    if key not in _PROGRAM:
        _PROGRAM[key] = _build_program(*key)
    return _PROGRAM[key]


def _round_fp32r(x):
    """Round fp32 to the PE's float32r format: round-to-nearest-even keeping
    11 explicit mantissa bits (low 12 bits zeroed)."""
    u = np.ascontiguousarray(x, dtype=np.float32).view(np.uint32)
    lsb = (u >> 12) & 1
    r = (u + np.uint32(0x7FF) + lsb) & np.uint32(0xFFFFF000)
    return r.view(np.float32)


def _host_dt(dt_mode):
    if dt_mode == "fp16":
        return np.float16
    if dt_mode == "fp32r":
        return np.float32
    from ml_dtypes import bfloat16

    return bfloat16


def _prep_inputs(support_set, query_set, dt_mode=None):
    dt_mode = dt_mode or DT_MODE
    S = np.asarray(support_set, dtype=np.float32)
    Q = np.asarray(query_set, dtype=np.float32)
    assert S.shape == (NN, DN) and Q.shape == (QN, DN)
    hdt = _host_dt(dt_mode)

    def normalize(x):
        x64 = x.astype(np.float64)
        norm = np.sqrt(np.einsum("nd,nd->n", x64, x64))
        # Reference divides by max(|q|*|s|, eps). Norms here are ~22, so the
        # eps clamp never binds for real rows; an all-zero row would give
        # dots == 0 in the reference too, so map inv-norm to 0 there.
        inv = np.where(norm > 0, 1.0 / np.maximum(norm, EPS), 0.0)
        return x64 * inv[:, None]

    Sn = normalize(S)
    Qn = normalize(Q)
    qT = np.ascontiguousarray(Qn.T).astype(hdt)  # [512, 2048]
    if dt_mode == "fp32r":
        qT = _round_fp32r(qT)
    in_maps = []
    for c in range(N_CORES):
        sT = np.zeros((DN, NSHP), dtype=hdt)
        sT[:, :NSH] = np.ascontiguousarray(Sn[c * NSH : (c + 1) * NSH].T).astype(
            hdt
        )
        if dt_mode == "fp32r":
            sT = _round_fp32r(sT)
        in_maps.append({"qT": qT, "sT": sT})
    return in_maps


def _run(in_maps, dt_mode=None, out_mode=None, trace=False, **kwargs):
    from concourse import bass_utils

    nc = _get_program(dt_mode, out_mode)
    return bass_utils.run_bass_kernel_spmd(
        nc, in_maps, core_ids=list(range(N_CORES)), trace=trace, **kwargs
    )


def _assemble(results):
    out = np.empty((QN, NN), dtype=np.float32)
    for c in range(N_CORES):
        blk = np.asarray(results[c]["out"])[:NSH]  # [6250, 2048]
        out[:, c * NSH : (c + 1) * NSH] = blk.T
    return out


def kernel(support_set, query_set):
    in_maps = _prep_inputs(support_set, query_set)
    res = _run(in_maps)
    return _assemble(res.results)


# revision 21
# speedup vs baseline: 1.0413x; 1.0413x over previous
"""Cosine-similarity retrieval kernel for Trainium2 (8 NeuronCores, SPMD).

Computes out[q, n] = cos(query[q], support[n]) for query [2048, 512] and
support [50000, 512], out [2048, 50000] float32 — matching
torch.nn.CosineSimilarity semantics (dots / max(|q|*|s|, 1e-8)).

Strategy:
  * Shard support on the N axis: 8 shards of 6250 rows (zero-padded to 6272 =
    49 blocks of 128). Each core reads its shard plus the replicated query
    set and writes its own [6272, 2048] output block (n-major, i.e. the
    transpose of the final layout); the host trims/transposes/concatenates —
    no device collective needed.
  * Rows are pre-normalized on the host (norms in float64), so the device
    kernel is a pure matmul; the PSUM result IS the cosine.
  * Storage/matmul dtype is fp16 (1 cycle/row on the PE, same as fp32r, but
    weights go through the LDWEIGHTS+FWL path instead of per-matmul fp32
    self-loading). The support block [128d, 128n] is the STATIONARY operand,
    reused across 4 consecutive matmuls that stream the resident query set
    512 columns at a time; with walrus --enable-ldw-opt the LDWEIGHTS for
    repeats is deduped, so weight-load overhead amortizes 4x and prefetches
    into the PE background buffer during the preceding matmuls.
  * PSUM: 4 banks accumulate one n-block over the 4 k-slices (bank = [128,
    512] fp32 = exactly one 2KB bank); the other 4 banks drain the previous
    n-block through ACT/DVE fp32->fp16 copies, so the PE never waits.
  * Output staged fp16 (halves the dominant HBM write traffic; host upcasts;
    ~2.4e-4 extra rel-L2). One store per n-block: 4KB-contiguous per
    partition, and the final store is only 0.5MB so the kernel-exit barrier
    isn't stuck behind a big trailing DMA.
"""

import os

import numpy as np

QN, DN, NN = 2048, 512, 50000
N_CORES = 8
NSH = NN // N_CORES  # 6250 support rows per core
P = 128
KT = DN // P  # 4 contraction slices
NBLK = (NSH + P - 1) // P  # 49 n-blocks per core
NSHP = NBLK * P  # 6272 (22 zero-padded rows, trimmed on host)
QC = 4  # query chunks, each one PSUM bank wide
QW = QN // QC  # 512 fp32 = one full PSUM bank
# n-blocks per DMA slab: small first slab so the first matmul unblocks after
# ~0.3MB of DMA; 1MB slabs after that for 2KB-contiguous packets.
SLAB_BLOCKS = [2, 8, 8, 8, 8, 8, 7]
SLAB_PREFETCH = 3
EPS = 1e-8

# "fp16" (default), "bf16", or "fp32r": SBUF/DMA storage + matmul dtype.
DT_MODE = os.environ.get("COS_DT_MODE", "fp16")
# Output staging dtype: "fp16" (default) or "fp32".
OUT_MODE = os.environ.get("COS_OUT_DT", "fp16")

_PROGRAM = {}


def _patch_ldw_opt():
    """walrus's LDWEIGHTS dedup (--enable-ldw-opt) is hardcoded off in
    concourse; consecutive matmuls here share weights, so turn it on."""
    from concourse import bass_utils as bu

    if getattr(bu.run_command, "_ldw_patched", False):
        return
    orig = bu.run_command

    def patched(argv, **kwargs):
        if isinstance(argv, list) and "--enable-ldw-opt=false" in argv:
            argv = [
                "--enable-ldw-opt=true" if a == "--enable-ldw-opt=false" else a
                for a in argv
            ]
        return orig(argv, **kwargs)

    patched._ldw_patched = True
    bu.run_command = patched


def _build_program(dt_mode, out_mode):
    import concourse.bass as bass  # noqa: F401
    import concourse.tile as tile
    from concourse import bacc, mybir

    # walrus's LDWEIGHTS dedup pass rejects fp16/bf16 (FWL-format) weight
    # loads outright ("InstLdweights is not compatible with LDW
    # optimization"), so only enable it for the fp32r fallback.  The fp16
    # per-matmul LDWEIGHTS is hidden by FWL + the PE's 64-deep reorder
    # window anyway (measured ~5ns/matmul exposure).
    if dt_mode == "fp32r" and os.environ.get("COS_LDW_OPT", "1") != "0":
        _patch_ldw_opt()

    store_dt = {
        "fp16": mybir.dt.float16,
        "bf16": mybir.dt.bfloat16,
        "fp32r": mybir.dt.float32r,
    }[dt_mode]
    out_dt = mybir.dt.float16 if out_mode == "fp16" else mybir.dt.float32

    nc = bacc.Bacc(
        "TRN2", target_bir_lowering=False, debug=False, num_devices=N_CORES
    )
    qT = nc.dram_tensor("qT", [DN, QN], store_dt, kind="ExternalInput").ap()
    sT = nc.dram_tensor("sT", [DN, NSHP], store_dt, kind="ExternalInput").ap()
    out = nc.dram_tensor("out", [NSHP, QN], out_dt, kind="ExternalOutput").ap()

    qT3 = qT.rearrange("(k p) q -> p k q", p=P)  # [128, KT, QN]
    sT3 = sT.rearrange("(k p) n -> p k n", p=P)  # [128, KT, NSHP]
    out3 = out.rearrange("(s p) q -> p s q", p=P)  # [128, NBLK, QN]

    slab_off, o = [], 0
    for nb in SLAB_BLOCKS:
        slab_off.append(o)
        o += nb
    assert o == NBLK

    with tile.TileContext(nc) as tc:
        with (
            tc.tile_pool(name="qw", bufs=1) as qpool,
            tc.tile_pool(name="sw", bufs=SLAB_PREFETCH + 1) as spool,
            tc.tile_pool(name="ps", bufs=8, space="PSUM") as pspool,
            tc.tile_pool(name="ostage", bufs=4) as opool,
        ):
            qt = qpool.tile([P, KT, QN], store_dt, name="qt", tag="qt")
            slabs = {}

            def load_slab(si, per_k=False):
                nb = SLAB_BLOCKS[si]
                n0 = slab_off[si] * P
                w = nb * P
                t = spool.tile(
                    [P, KT, 8 * P],
                    store_dt,
                    name=f"s{si}",
                    tag="ss",
                    bufs=SLAB_PREFETCH + 1,
                )
                if per_k:
                    for k in range(KT):
                        nc.sync.dma_start(t[:, k, :w], sT3[:, k, n0 : n0 + w])
                else:
                    nc.sync.dma_start(t[:, :, :w], sT3[:, :, n0 : n0 + w])
                slabs[si] = t

            # Startup: everything stays on the sync queue (spreading over
            # other engines' queues loses: each ring pays its own multi-us
            # cold ramp, and the extra DMA semaphores lengthen the exit
            # drain/clear epilogue).  s0 k-slices interleave with the query
            # k-slices in first-block need-order; the startup is then purely
            # DMA-bandwidth-bound (~2.3MB before the first full block pass).
            nb0 = SLAB_BLOCKS[0]
            w0 = nb0 * P
            t0s = spool.tile(
                [P, KT, 8 * P],
                store_dt,
                name="s0",
                tag="ss",
                bufs=SLAB_PREFETCH + 1,
            )
            for k in range(KT):
                nc.sync.dma_start(t0s[:, k, :w0], sT3[:, k, 0:w0])
                nc.sync.dma_start(qt[:, k, :], qT3[:, k, :])
            slabs[0] = t0s
            for si in range(1, SLAB_PREFETCH):
                load_slab(si)

            copy_idx = 0
            for si, nb in enumerate(SLAB_BLOCKS):
                if si + SLAB_PREFETCH < len(SLAB_BLOCKS):
                    load_slab(si + SLAB_PREFETCH)
                for b in range(nb):
                    sb = slab_off[si] + b
                    pss = [
                        pspool.tile(
                            [P, QW], mybir.dt.float32, name="ps", tag="ps"
                        )
                        for _ in range(QC)
                    ]
                    ot = opool.tile([P, QN], out_dt, name="ot", tag="ot")
                    # k outer / qc inner: the 4 qc matmuls stream against
                    # one stationary [128, 128] support block, so its
                    # LDWEIGHTS prefetches into the PE background buffer
                    # during the previous k's matmuls.
                    for k in range(KT):
                        wt = slabs[si][:, k, b * P : (b + 1) * P]
                        for qc in range(QC):
                            nc.tensor.matmul(
                                pss[qc][:, :],
                                lhsT=wt,
                                rhs=qt[:, k, qc * QW : (qc + 1) * QW],
                                start=(k == 0),
                                stop=(k == KT - 1),
                            )
                    # split PSUM->SBUF downcast copies across ACT/DVE
                    for qc in range(QC):
                        dst = ot[:, qc * QW : (qc + 1) * QW]
                        if copy_idx % 2 == 0:
                            nc.scalar.copy(out=dst, in_=pss[qc][:, :])
                        else:
                            nc.vector.tensor_copy(out=dst, in_=pss[qc][:, :])
                        copy_idx += 1
                    nc.sync.dma_start(out3[:, sb, :], ot[:, :])
    nc.compile()
    return nc


def _get_program(dt_mode=None, out_mode=None):
    key = (dt_mode or DT_MODE, out_mode or OUT_MODE)
    if key not in _PROGRAM:
        _PROGRAM[key] = _build_program(*key)
    return _PROGRAM[key]


def _round_fp32r(x):
    """Round fp32 to the PE's float32r format: round-to-nearest-even keeping
    11 explicit mantissa bits (low 12 bits zeroed)."""
    u = np.ascontiguousarray(x, dtype=np.float32).view(np.uint32)
    lsb = (u >> 12) & 1
    r = (u + np.uint32(0x7FF) + lsb) & np.uint32(0xFFFFF000)
    return r.view(np.float32)


def _host_dt(dt_mode):
    if dt_mode == "fp16":
        return np.float16
    if dt_mode == "fp32r":
        return np.float32
    from ml_dtypes import bfloat16

    return bfloat16


def _prep_inputs(support_set, query_set, dt_mode=None):
    dt_mode = dt_mode or DT_MODE
    S = np.asarray(support_set, dtype=np.float32)
    Q = np.asarray(query_set, dtype=np.float32)
    assert S.shape == (NN, DN) and Q.shape == (QN, DN)
    hdt = _host_dt(dt_mode)

    def normalize(x):
        x64 = x.astype(np.float64)
        norm = np.sqrt(np.einsum("nd,nd->n", x64, x64))
        # Reference divides by max(|q|*|s|, eps). Norms here are ~22, so the
        # eps clamp never binds for real rows; an all-zero row would give
        # dots == 0 in the reference too, so map inv-norm to 0 there.
        inv = np.where(norm > 0, 1.0 / np.maximum(norm, EPS), 0.0)
        return x64 * inv[:, None]

    Sn = normalize(S)
    Qn = normalize(Q)
    qT = np.ascontiguousarray(Qn.T).astype(hdt)  # [512, 2048]
    if dt_mode == "fp32r":
        qT = _round_fp32r(qT)
    in_maps = []
    for c in range(N_CORES):
        sT = np.zeros((DN, NSHP), dtype=hdt)
        sT[:, :NSH] = np.ascontiguousarray(Sn[c * NSH : (c + 1) * NSH].T).astype(
            hdt
        )
        if dt_mode == "fp32r":
            sT = _round_fp32r(sT)
        in_maps.append({"qT": qT, "sT": sT})
    return in_maps


def _run(in_maps, dt_mode=None, out_mode=None, trace=False, **kwargs):
    from concourse import bass_utils

    nc = _get_program(dt_mode, out_mode)
    return bass_utils.run_bass_kernel_spmd(
        nc, in_maps, core_ids=list(range(N_CORES)), trace=trace, **kwargs
    )


def _assemble(results):
    out = np.empty((QN, NN), dtype=np.float32)
    for c in range(N_CORES):
        blk = np.asarray(results[c]["out"])[:NSH]  # [6250, 2048]
        out[:, c * NSH : (c + 1) * NSH] = blk.T
    return out


def kernel(support_set, query_set):
    in_maps = _prep_inputs(support_set, query_set)
    res = _run(in_maps)
    return _assemble(res.results)


# revision 23
# speedup vs baseline: 1.0416x; 1.0003x over previous
"""Cosine-similarity retrieval kernel for Trainium2 (8 NeuronCores, SPMD).

Computes out[q, n] = cos(query[q], support[n]) for query [2048, 512] and
support [50000, 512], out [2048, 50000] float32 — matching
torch.nn.CosineSimilarity semantics (dots / max(|q|*|s|, 1e-8)).

Strategy:
  * Shard support on the N axis: 8 shards of 6250 rows (zero-padded to 6272 =
    49 blocks of 128). Each core reads its shard plus the replicated query
    set and writes its own [6272, 2048] output block (n-major, i.e. the
    transpose of the final layout); the host trims/transposes/concatenates —
    no device collective needed.
  * Rows are pre-normalized on the host (norms in float64), so the device
    kernel is a pure matmul; the PSUM result IS the cosine.
  * Storage/matmul dtype is fp16 (1 cycle/row on the PE, same as fp32r, but
    weights go through the LDWEIGHTS+FWL path instead of per-matmul fp32
    self-loading). The support block [128d, 128n] is the STATIONARY operand,
    reused across 4 consecutive matmuls that stream the resident query set
    512 columns at a time; with walrus --enable-ldw-opt the LDWEIGHTS for
    repeats is deduped, so weight-load overhead amortizes 4x and prefetches
    into the PE background buffer during the preceding matmuls.
  * PSUM: 4 banks accumulate one n-block over the 4 k-slices (bank = [128,
    512] fp32 = exactly one 2KB bank); the other 4 banks drain the previous
    n-block through ACT/DVE fp32->fp16 copies, so the PE never waits.
  * Output staged fp16 (halves the dominant HBM write traffic; host upcasts;
    ~2.4e-4 extra rel-L2). One store per n-block: 4KB-contiguous per
    partition, and the final store is only 0.5MB so the kernel-exit barrier
    isn't stuck behind a big trailing DMA.
"""

import os

import numpy as np

QN, DN, NN = 2048, 512, 50000
N_CORES = 8
NSH = NN // N_CORES  # 6250 support rows per core
P = 128
KT = DN // P  # 4 contraction slices
NBLK = (NSH + P - 1) // P  # 49 n-blocks per core
NSHP = NBLK * P  # 6272 (22 zero-padded rows, trimmed on host)
QC = 4  # query chunks, each one PSUM bank wide
QW = QN // QC  # 512 fp32 = one full PSUM bank
# n-blocks per DMA slab: small first slab so the first matmul unblocks after
# ~0.3MB of DMA; 1MB slabs after that for 2KB-contiguous packets.
SLAB_BLOCKS = [2, 8, 8, 8, 8, 8, 7]
SLAB_PREFETCH = 3
EPS = 1e-8

# "fp16" (default), "bf16", or "fp32r": SBUF/DMA storage + matmul dtype.
DT_MODE = os.environ.get("COS_DT_MODE", "fp16")
# Output staging dtype: "fp16" (default) or "fp32".
OUT_MODE = os.environ.get("COS_OUT_DT", "fp16")

_PROGRAM = {}


def _patch_ldw_opt():
    """walrus's LDWEIGHTS dedup (--enable-ldw-opt) is hardcoded off in
    concourse; consecutive matmuls here share weights, so turn it on."""
    from concourse import bass_utils as bu

    if getattr(bu.run_command, "_ldw_patched", False):
        return
    orig = bu.run_command

    def patched(argv, **kwargs):
        if isinstance(argv, list) and "--enable-ldw-opt=false" in argv:
            argv = [
                "--enable-ldw-opt=true" if a == "--enable-ldw-opt=false" else a
                for a in argv
            ]
        return orig(argv, **kwargs)

    patched._ldw_patched = True
    bu.run_command = patched


def _build_program(dt_mode, out_mode):
    import concourse.bass as bass  # noqa: F401
    import concourse.tile as tile
    from concourse import bacc, mybir

    # walrus's LDWEIGHTS dedup pass rejects fp16/bf16 (FWL-format) weight
    # loads outright ("InstLdweights is not compatible with LDW
    # optimization"), so only enable it for the fp32r fallback.  The fp16
    # per-matmul LDWEIGHTS is hidden by FWL + the PE's 64-deep reorder
    # window anyway (measured ~5ns/matmul exposure).
    if dt_mode == "fp32r" and os.environ.get("COS_LDW_OPT", "1") != "0":
        _patch_ldw_opt()

    store_dt = {
        "fp16": mybir.dt.float16,
        "bf16": mybir.dt.bfloat16,
        "fp32r": mybir.dt.float32r,
    }[dt_mode]
    out_dt = mybir.dt.float16 if out_mode == "fp16" else mybir.dt.float32

    nc = bacc.Bacc(
        "TRN2", target_bir_lowering=False, debug=False, num_devices=N_CORES
    )
    qT = nc.dram_tensor("qT", [DN, QN], store_dt, kind="ExternalInput").ap()
    sT = nc.dram_tensor("sT", [DN, NSHP], store_dt, kind="ExternalInput").ap()
    out = nc.dram_tensor("out", [NSHP, QN], out_dt, kind="ExternalOutput").ap()

    qT3 = qT.rearrange("(k p) q -> p k q", p=P)  # [128, KT, QN]
    sT3 = sT.rearrange("(k p) n -> p k n", p=P)  # [128, KT, NSHP]
    out3 = out.rearrange("(s p) q -> p s q", p=P)  # [128, NBLK, QN]

    slab_off, o = [], 0
    for nb in SLAB_BLOCKS:
        slab_off.append(o)
        o += nb
    assert o == NBLK

    with tile.TileContext(nc) as tc:
        with (
            tc.tile_pool(name="qw", bufs=1) as qpool,
            tc.tile_pool(name="sw", bufs=SLAB_PREFETCH + 1) as spool,
            tc.tile_pool(name="ps", bufs=8, space="PSUM") as pspool,
            tc.tile_pool(name="ostage", bufs=4) as opool,
        ):
            qt = qpool.tile([P, KT, QN], store_dt, name="qt", tag="qt")
            slabs = {}

            def load_slab(si, per_k=False):
                nb = SLAB_BLOCKS[si]
                n0 = slab_off[si] * P
                w = nb * P
                t = spool.tile(
                    [P, KT, 8 * P],
                    store_dt,
                    name=f"s{si}",
                    tag="ss",
                    bufs=SLAB_PREFETCH + 1,
                )
                if per_k:
                    for k in range(KT):
                        nc.sync.dma_start(t[:, k, :w], sT3[:, k, n0 : n0 + w])
                else:
                    nc.sync.dma_start(t[:, :, :w], sT3[:, :, n0 : n0 + w])
                slabs[si] = t

            # Startup: everything stays on the sync queue (spreading over
            # other engines' queues loses: each ring pays its own multi-us
            # cold ramp, and the extra DMA semaphores lengthen the exit
            # drain/clear epilogue).  s0 k-slices interleave with the query
            # k-slices in first-block need-order; the startup is then purely
            # DMA-bandwidth-bound (~2.3MB before the first full block pass).
            nb0 = SLAB_BLOCKS[0]
            w0 = nb0 * P
            t0s = spool.tile(
                [P, KT, 8 * P],
                store_dt,
                name="s0",
                tag="ss",
                bufs=SLAB_PREFETCH + 1,
            )
            for k in range(KT):
                # q slice first: it is 8x bigger than the s0 slice, so the
                # first matmul is gated on it — give it the queue head start
                nc.sync.dma_start(qt[:, k, :], qT3[:, k, :])
                nc.sync.dma_start(t0s[:, k, :w0], sT3[:, k, 0:w0])
            slabs[0] = t0s
            for si in range(1, SLAB_PREFETCH):
                load_slab(si)

            copy_idx = 0
            for si, nb in enumerate(SLAB_BLOCKS):
                if si + SLAB_PREFETCH < len(SLAB_BLOCKS):
                    load_slab(si + SLAB_PREFETCH)
                for b in range(nb):
                    sb = slab_off[si] + b
                    pss = [
                        pspool.tile(
                            [P, QW], mybir.dt.float32, name="ps", tag="ps"
                        )
                        for _ in range(QC)
                    ]
                    ot = opool.tile([P, QN], out_dt, name="ot", tag="ot")
                    # k outer / qc inner: the 4 qc matmuls stream against
                    # one stationary [128, 128] support block, so its
                    # LDWEIGHTS prefetches into the PE background buffer
                    # during the previous k's matmuls.
                    for k in range(KT):
                        wt = slabs[si][:, k, b * P : (b + 1) * P]
                        for qc in range(QC):
                            nc.tensor.matmul(
                                pss[qc][:, :],
                                lhsT=wt,
                                rhs=qt[:, k, qc * QW : (qc + 1) * QW],
                                start=(k == 0),
                                stop=(k == KT - 1),
                            )
                    # split PSUM->SBUF downcast copies across ACT/DVE
                    for qc in range(QC):
                        dst = ot[:, qc * QW : (qc + 1) * QW]
                        if copy_idx % 2 == 0:
                            nc.scalar.copy(out=dst, in_=pss[qc][:, :])
                        else:
                            nc.vector.tensor_copy(out=dst, in_=pss[qc][:, :])
                        copy_idx += 1
                    if sb < NBLK - 1:
                        nc.sync.dma_start(out3[:, sb, :], ot[:, :])
                    else:
                        # final block: store in two halves, each issued as
                        # soon as its pair of copies lands, so the exit
                        # barrier waits on a pipelined 0.25MB trailing store
                        # instead of the full block after all four copies
                        h = QN // 2
                        nc.sync.dma_start(out3[:, sb, :h], ot[:, :h])
                        nc.sync.dma_start(out3[:, sb, h:], ot[:, h:])
    nc.compile()
    return nc


def _get_program(dt_mode=None, out_mode=None):
    key = (dt_mode or DT_MODE, out_mode or OUT_MODE)
    if key not in _PROGRAM:
        _PROGRAM[key] = _build_program(*key)
    return _PROGRAM[key]


def _round_fp32r(x):
    """Round fp32 to the PE's float32r format: round-to-nearest-even keeping
    11 explicit mantissa bits (low 12 bits zeroed)."""
    u = np.ascontiguousarray(x, dtype=np.float32).view(np.uint32)
    lsb = (u >> 12) & 1
    r = (u + np.uint32(0x7FF) + lsb) & np.uint32(0xFFFFF000)
    return r.view(np.float32)


def _host_dt(dt_mode):
    if dt_mode == "fp16":
        return np.float16
    if dt_mode == "fp32r":
        return np.float32
    from ml_dtypes import bfloat16

    return bfloat16


def _prep_inputs(support_set, query_set, dt_mode=None):
    dt_mode = dt_mode or DT_MODE
    S = np.asarray(support_set, dtype=np.float32)
    Q = np.asarray(query_set, dtype=np.float32)
    assert S.shape == (NN, DN) and Q.shape == (QN, DN)
    hdt = _host_dt(dt_mode)

    def normalize(x):
        x64 = x.astype(np.float64)
        norm = np.sqrt(np.einsum("nd,nd->n", x64, x64))
        # Reference divides by max(|q|*|s|, eps). Norms here are ~22, so the
        # eps clamp never binds for real rows; an all-zero row would give
        # dots == 0 in the reference too, so map inv-norm to 0 there.
        inv = np.where(norm > 0, 1.0 / np.maximum(norm, EPS), 0.0)
        return x64 * inv[:, None]

    Sn = normalize(S)
    Qn = normalize(Q)
    qT = np.ascontiguousarray(Qn.T).astype(hdt)  # [512, 2048]
    if dt_mode == "fp32r":
        qT = _round_fp32r(qT)
    in_maps = []
    for c in range(N_CORES):
        sT = np.zeros((DN, NSHP), dtype=hdt)
        sT[:, :NSH] = np.ascontiguousarray(Sn[c * NSH : (c + 1) * NSH].T).astype(
            hdt
        )
        if dt_mode == "fp32r":
            sT = _round_fp32r(sT)
        in_maps.append({"qT": qT, "sT": sT})
    return in_maps


def _run(in_maps, dt_mode=None, out_mode=None, trace=False, **kwargs):
    from concourse import bass_utils

    nc = _get_program(dt_mode, out_mode)
    return bass_utils.run_bass_kernel_spmd(
        nc, in_maps, core_ids=list(range(N_CORES)), trace=trace, **kwargs
    )


def _assemble(results):
    out = np.empty((QN, NN), dtype=np.float32)
    for c in range(N_CORES):
        blk = np.asarray(results[c]["out"])[:NSH]  # [6250, 2048]
        out[:, c * NSH : (c + 1) * NSH] = blk.T
    return out


def kernel(support_set, query_set):
    in_maps = _prep_inputs(support_set, query_set)
    res = _run(in_maps)
    return _assemble(res.results)
